# revision 2
# baseline (speedup 1.0000x reference)
"""Trainium2 Bass kernel for nn_BiLSTM_2491081031886.

Single-layer unidirectional LSTM (B=2048, T=256, F=H=128) + Linear([T*H]->1).
Data-parallel over 8 NeuronCores: each core owns a 256-row batch shard and
runs the full sequential scan locally; weights are replicated.

Per-core dataflow (v2, decoupled waves), all layouts [hidden, batch]:
  - x is pre-transposed and cast to bf16 on the host ([F, T, BS] per shard),
    so chunked plain DMA lands it directly in the matmul-ready layout.
  - Gate pre-activations accumulate in PSUM as 4 blocks [i|f|o|2g] x 256 cols
    per step, in three bank-aligned rotating buffers (cols 0/1024/2048):
      bias (K=4 matmul vs a block-indicator, bf16)
      + W_ih^T.T @ xT_t (bf16, N=256)
      + W_hh2^T.T @ h_half (bf16, N=128 per wave)
  - Three independent batch waves (cols 0:86/86:171/171:256) run the
    recurrence with no cross-wave data dependency; their serial chains
    interleave on the engines, hiding the per-step matmul->sigmoid->DVE
    dependency-ring latency.  Per wave and step:
      sg    = Sigmoid(blocks)                   # one packed ACT op, bf16 out
      t2h   = (sg2g - 0.5) * sgi                # DVE stt, bf16
      u     = sgf * cd_prev                     # DVE tt (cd fp32 SBUF)
      cd    = 4*t2h + u                         # DVE stt, fp32
      scd   = Sigmoid(cd)                       # small ACT op, bf16
      h_half= (scd - 0.5) * sgo                 # DVE stt, bf16
    with tanh realized via sigmoid (weights pre-scaled x2 on the g-chunk,
    doubled cell state cd = 2c) and the h/2 factor absorbed into 2x on
    W_hh and w_lin.
  - Output head: acc[1,wave] += (2*w_lin_t) as lhsT against h_half (bf16),
    accumulated in PSUM over all steps; +b_lin on host.  The three wlin
    accumulators live in separate PSUM banks (4/5/6): a start=True matmul
    resets has_written for its whole bank, so they must not share one.
"""

import numpy as np
import ml_dtypes

import concourse.bacc as bacc
import concourse.bass as bass
import concourse.mybir as mybir
from concourse import tile
from concourse.bass_utils import run_bass_kernel_spmd

F32 = mybir.dt.float32
BF16 = mybir.dt.bfloat16
AF = mybir.ActivationFunctionType
OP = mybir.AluOpType

B, T_FULL, F = 2048, 256, 128
H = F
NCORES = 8
BS = B // NCORES  # 256 batch rows per core
WAVES = (("A", 0, 64), ("B", 64, 64), ("C", 128, 64), ("D", 192, 64))
W2 = 128
TC = 8            # timesteps per x-ingest chunk

# PSUM column layout (fp32 words per partition, 4096 total = 8 banks x 512)
PS_BUF = (0, 1024)     # two step buffers, 2 banks each (banks 0-3)
BLK = 256              # block width: [i|f|o|2g] each 256 cols (A:0-127 B:128+)
WLIN = {"A": 2048, "B": 2560, "C": 3072}  # banks 4/5/6, one per wave


def build(T=T_FULL, ablate=(), period_ns=0.0, waves=WAVES):
    """ablate: timing-only experiment knobs (results become wrong):
    'wlin' drop output-head matmuls; 'xin' drop x ingest (static xT);
    'dve' drop cell-math DVE ops; 'bias' drop bias matmuls; 'act' drop
    sigma ops; 'rec' drop recurrent matmuls.
    period_ns > 0 paces the scan at that many ns per step via virtual
    not-before times (tile_wait_until), phase-shifting the two waves."""
    ablate = set(ablate)
    assert T % 2 == 0
    nc = bacc.Bacc("TRN2", target_bir_lowering=False, debug=False,
                   num_devices=NCORES)

    xt_d = nc.dram_tensor("xt", [F, T, BS], BF16, kind="ExternalInput")
    whh_d = nc.dram_tensor("whh", [H, 4 * H], BF16, kind="ExternalInput")
    wih_d = nc.dram_tensor("wih", [F, 4 * H], BF16, kind="ExternalInput")
    b4_d = nc.dram_tensor("b4", [4, H], BF16, kind="ExternalInput")
    e4_d = nc.dram_tensor("e4", [4, 1024], BF16, kind="ExternalInput")
    wl_d = nc.dram_tensor("wl", [H, T], BF16, kind="ExternalInput")
    out_d = nc.dram_tensor("out", [BS], F32, kind="ExternalOutput")

    n_chunks = (T + TC - 1) // TC
    no_dve = "dve" in ablate
    no_act = "act" in ablate

    with tile.TileContext(nc) as tc:
        with (
            tc.tile_pool(name="const", bufs=1) as constp,
            tc.tile_pool(name="xT", bufs=3) as xtp,
            tc.tile_pool(name="sig", bufs=3) as sigp,
            tc.tile_pool(name="hh", bufs=3) as hhp,
            tc.tile_pool(name="cd", bufs=3) as cdp,
            tc.tile_pool(name="tmp", bufs=3) as tmpp,
            tc.tile_pool(name="psum", bufs=1, space=bass.MemorySpace.PSUM) as psp,
        ):
            # ---- constants ----
            whh = constp.tile([H, 4 * H], BF16)
            wih = constp.tile([F, 4 * H], BF16)
            b4 = constp.tile([4, H], BF16)
            e4 = constp.tile([4, 1024], BF16)
            wl = constp.tile([H, T], BF16)
            nc.sync.dma_start(whh[:], whh_d.ap())
            nc.sync.dma_start(wih[:], wih_d.ap())
            nc.sync.dma_start(b4[:], b4_d.ap())
            nc.sync.dma_start(e4[:], e4_d.ap())
            nc.sync.dma_start(wl[:], wl_d.ap())

            ps = psp.tile([128, 4096], F32)

            # ---- x ingest: host pre-transposed [F, T, BS] bf16, plain
            # chunked DMA straight into the matmul-ready layout ----
            xtap = xt_d.ap()
            xchunks = []
            if "xin" in ablate:
                x0 = constp.tile([F, TC, BS], BF16)
                nc.sync.dma_start(x0[:], xtap[:, 0:TC, :])
                xchunks = [x0] * n_chunks
            else:
                for ch in range(n_chunks):
                    t0 = ch * TC
                    tc_n = min(TC, T - t0)
                    xc = xtp.tile([F, TC, BS], BF16)
                    nc.sync.dma_start(xc[:, 0:tc_n, :], xtap[:, t0:t0 + tc_n, :])
                    xchunks.append(xc)

            def make_xt(t):
                return xchunks[t // TC][:, t % TC, :]

            # ---- state tiles ----
            # wlin accumulators: pack waves two-per-bank in banks 4-7. Each
            # bank is zeroed by one ones^T@zeros matmul (start=True) before
            # the scan; the RAW dep on those cols orders it before every
            # wlin accumulate (all start=False).
            wl_bank = {}
            used_banks = set()
            for i, (w, co, wn) in enumerate(waves):
                wl_bank[w] = 2048 + 256 * i
                used_banks.add(2048 + 512 * (i // 2))
            zrow = constp.tile([H, 512], BF16)
            onesc = constp.tile([H, 1], BF16)
            nc.vector.memset(zrow[:], 0.0)
            nc.vector.memset(onesc[:], 1.0)
            for bk in sorted(used_banks):
                nc.tensor.matmul(ps[0:1, bk:bk + 512], onesc[:], zrow[:],
                                 start=True, stop=False,
                                 skip_group_check=True)
            wv_w = {w: n for w, _, n in waves}
            cd_prev = {}
            for w, _, wn in waves:
                t0c = cdp.tile([H, wn], F32, tag=f"cd{w}")
                nc.vector.memset(t0c[:], 0.0)
                cd_prev[w] = t0c
            if no_dve:
                hh_s = {}
                for w, _, wn in waves:
                    hh_s[w] = hhp.tile([H, wn], BF16, tag=f"hh{w}")
                    nc.vector.memset(hh_s[w][:], 0.01)
            if no_act:
                s_s = {}
                for w, _, wn in waves:
                    s_s[w] = sigp.tile([128, 4, wn], BF16, tag=f"s{w}")
                    nc.vector.memset(s_s[w][:], 0.5)
                scd_s = tmpp.tile([H, 86], BF16, tag="scds")
                nc.vector.memset(scd_s[:], 0.5)

            hh_prev = {w: None for w, _, _ in waves}
            s_last = {w: None for w, _, _ in waves}
            wv_co = {w: c for w, c, _ in waves}

            cd_cur = {}

            def wave_p1(t, w):
                """Recurrent matmuls + the packed gates sigma."""
                base = PS_BUF[t % 2]
                co, wn = wv_co[w], wv_w[w]
                if hh_prev[w] is not None and "rec" not in ablate:
                    for c in range(4):
                        nc.tensor.matmul(
                            ps[:, base + c * BLK + co:base + c * BLK + co + wn],
                            whh[:, c * H:(c + 1) * H], hh_prev[w][:],
                            start=False, stop=False, skip_group_check=True)
                blocks = ps[:, base:base + 1024].rearrange(
                    "p (c n) -> p c n", c=4)
                if no_act:
                    s = s_s[w]
                else:
                    s = sigp.tile([128, 4, wn], BF16, tag=f"s{w}")
                    nc.scalar.activation(s[:], blocks[:, :, co:co + wn],
                                         AF.Sigmoid)
                s_last[w] = s

            def wave_p2(t, w):
                """Cell-state DVE chain: t2h, u, cd."""
                if no_dve:
                    return
                s = s_last[w]
                wn = wv_w[w]
                t2h = tmpp.tile([H, wn], BF16, tag=f"t2h{w}")
                nc.vector.scalar_tensor_tensor(
                    t2h[:], s[:, 3, :], -0.5, s[:, 0, :], OP.add, OP.mult)
                u = tmpp.tile([H, wn], F32, tag=f"u{w}")
                nc.vector.tensor_tensor(u[:], s[:, 1, :], cd_prev[w][:],
                                        OP.mult)
                cd = cdp.tile([H, wn], F32, tag=f"cd{w}")
                nc.vector.scalar_tensor_tensor(
                    cd[:], t2h[:], 4.0, u[:], OP.mult, OP.add)
                cd_prev[w] = cd
                cd_cur[w] = cd

            def wave_p3(t, w):
                """tanh(c) (via scaled-Tanh ACT, same table set as Sigmoid),
                h = tanh(c)*sgo as a plain 2x-mode tensor_tensor, and the
                output-head matmul."""
                if no_dve:
                    hh_prev[w] = hh_s[w]
                    return
                s = s_last[w]
                wn = wv_w[w]
                if no_act:
                    scd = scd_s
                else:
                    scd = tmpp.tile([H, wn], BF16, tag=f"scd{w}")
                    nc.scalar.activation(scd[:], cd_cur[w][:], AF.Tanh,
                                         scale=0.5)
                hh = hhp.tile([H, wn], BF16, tag=f"hh{w}")
                nc.vector.tensor_tensor(hh[:], scd[:], s[:, 2, :], OP.mult)
                hh_prev[w] = hh
                # output head
                if "wlin" not in ablate:
                    acc = wl_bank[w]
                    nc.tensor.matmul(
                        ps[0:1, acc:acc + wn], wl[:, t:t + 1], hh[:],
                        start=False, stop=(t == T - 1),
                        skip_group_check=True)

            # ---- main scan: 3-phase wave rotation. Waves A/B/C run 1/3 step
            # apart so their ACT ops (sigma-gates, tanh-cd) round-robin on
            # the Activation engine without convoys. Steady-state slot t:
            #   xg(t) A.p1(t) C.p2(t-1) B.p3(t-1) B.p1(t) A.p2(t)
            #   C.p3(t-1) C.p1(t) B.p2(t) A.p3(t)
            first = "bias" in ablate

            def emit_xg(t):
                base = PS_BUF[t % 2]
                xt = make_xt(t)
                if "bias" not in ablate:
                    nc.tensor.matmul(
                        ps[:, base:base + 512], b4[:], e4[:, 0:512],
                        start=True, stop=False, skip_group_check=True)
                    nc.tensor.matmul(
                        ps[:, base + 512:base + 1024], b4[:],
                        e4[:, 512:1024],
                        start=True, stop=False, skip_group_check=True)
                for c in range(4):
                    nc.tensor.matmul(
                        ps[:, base + c * BLK:base + (c + 1) * BLK],
                        wih[:, c * H:(c + 1) * H], xt,
                        start=first, stop=False, skip_group_check=True)

            PH = {1: wave_p1, 2: wave_p2, 3: wave_p3}
            NW = len(waves)

            for slot in range(T + 1):
                if slot < T:
                    emit_xg(slot)
                ev = []
                for i, (w, _, _) in enumerate(waves):
                    for ph in (1, 2, 3):
                        # wave i runs phase ph of step t at virtual time
                        # t + i/NW + (ph-1)/3; emit the one landing in
                        # [slot, slot+1)
                        frac = i / NW + (ph - 1) / 3.0
                        t = slot - int(frac)
                        vt = frac - int(frac)
                        ev.append((vt, ph, t, w))
                ev.sort(key=lambda e: (e[0], -e[2]))
                for vt, ph, t, w in ev:
                    if 0 <= t < T:
                        PH[ph](t, w)

            # output
            outsb = constp.tile([1, 2 * W2], F32)
            for w, co, wn in waves:
                nc.vector.tensor_copy(outsb[0:1, co:co + wn],
                                      ps[0:1, wl_bank[w]:wl_bank[w] + wn])
            nc.sync.dma_start(out_d.ap().rearrange("(a b) -> a b", a=1),
                              outsb[:])

    nc.compile()
    return nc


_CACHE = {}


def _get_nc(T=T_FULL):
    if T not in _CACHE:
        _CACHE[T] = build(T)
    return _CACHE[T]


def prep_weights(w_ih, w_hh, b_ih, b_hh, w_lin, T=T_FULL):
    """Host-side weight prep. Chunk order [i, f, o, g]; g-chunk pre-scaled x2
    (sigmoid(2g) trick). hh on device holds full h (tanh-based), so W_hh and
    w_lin are NOT pre-scaled."""
    perm = np.r_[0:H, H:2 * H, 3 * H:4 * H, 2 * H:3 * H]
    gs = np.ones((4 * H, 1), np.float32)
    gs[3 * H:] = 2.0
    bf = ml_dtypes.bfloat16
    whh = np.ascontiguousarray((w_hh[perm] * gs).T.astype(bf))
    wih = np.ascontiguousarray((w_ih[perm] * gs).T.astype(bf))
    b4 = ((b_ih + b_hh)[perm] * gs[:, 0]).reshape(4, H).astype(bf)
    e4 = np.zeros((4, 1024), bf)
    for c in range(4):
        e4[c, c * 256:(c + 1) * 256] = 1.0
    wl = np.ascontiguousarray(w_lin.reshape(T, H).T.astype(bf))
    return whh, wih, b4, e4, wl


def prep_x(x):
    """Shard + host-transpose x to [F, T, BS] bf16 per core (the layout the
    xg matmuls consume, so no on-chip transpose is needed)."""
    xb = x.astype(ml_dtypes.bfloat16)
    return [np.ascontiguousarray(xb[c * BS:(c + 1) * BS].transpose(2, 1, 0))
            for c in range(NCORES)]


def kernel(x, w_ih, w_hh, b_ih, b_hh, w_lin, b_lin):
    x = np.asarray(x, np.float32)
    T = x.shape[1]
    nc = _get_nc(T)
    whh, wih, b4, e4, wl = prep_weights(
        np.asarray(w_ih, np.float32), np.asarray(w_hh, np.float32),
        np.asarray(b_ih, np.float32), np.asarray(b_hh, np.float32),
        np.asarray(w_lin, np.float32), T)
    xts = prep_x(x)
    in_maps = []
    for c in range(NCORES):
        in_maps.append({
            "xt": xts[c],
            "whh": whh, "wih": wih, "b4": b4, "e4": e4, "wl": wl,
        })
    res = run_bass_kernel_spmd(nc, in_maps, core_ids=list(range(NCORES)))
    out = np.concatenate([r["out"] for r in res.results])
    return (out + np.float32(b_lin[0])).astype(np.float32)



# revision 4
# speedup vs baseline: 1.2915x; 1.2915x over previous
"""Trainium2 Bass kernel for nn_BiLSTM_2491081031886.

Single-layer unidirectional LSTM (B=2048, T=256, F=H=128) + Linear([T*H]->1).
Data-parallel over 8 NeuronCores: each core owns a 256-row batch shard and
runs the full sequential scan locally; weights are replicated.

Per-core dataflow (v2, decoupled waves), all layouts [hidden, batch]:
  - x is pre-transposed and cast to bf16 on the host ([F, T, BS] per shard),
    so chunked plain DMA lands it directly in the matmul-ready layout.
  - Gate pre-activations accumulate in PSUM as 4 blocks [i|f|o|2g] x 256 cols
    per step, in three bank-aligned rotating buffers (cols 0/1024/2048):
      bias (K=4 matmul vs a block-indicator, bf16)
      + W_ih^T.T @ xT_t (bf16, N=256)
      + W_hh2^T.T @ h_half (bf16, N=128 per wave)
  - Three independent batch waves (cols 0:86/86:171/171:256) run the
    recurrence with no cross-wave data dependency; their serial chains
    interleave on the engines, hiding the per-step matmul->sigmoid->DVE
    dependency-ring latency.  Per wave and step:
      sg    = Sigmoid(blocks)                   # one packed ACT op, bf16 out
      t2h   = (sg2g - 0.5) * sgi                # DVE stt, bf16
      u     = sgf * cd_prev                     # DVE tt (cd fp32 SBUF)
      cd    = 4*t2h + u                         # DVE stt, fp32
      scd   = Sigmoid(cd)                       # small ACT op, bf16
      h_half= (scd - 0.5) * sgo                 # DVE stt, bf16
    with tanh realized via sigmoid (weights pre-scaled x2 on the g-chunk,
    doubled cell state cd = 2c) and the h/2 factor absorbed into 2x on
    W_hh and w_lin.
  - Output head: acc[1,wave] += (2*w_lin_t) as lhsT against h_half (bf16),
    accumulated in PSUM over all steps; +b_lin on host.  The three wlin
    accumulators live in separate PSUM banks (4/5/6): a start=True matmul
    resets has_written for its whole bank, so they must not share one.
"""

import numpy as np
import ml_dtypes

import concourse.bacc as bacc
import concourse.bass as bass
import concourse.mybir as mybir
from concourse import tile
from concourse.bass_utils import run_bass_kernel_spmd

F32 = mybir.dt.float32
BF16 = mybir.dt.bfloat16
AF = mybir.ActivationFunctionType
OP = mybir.AluOpType

B, T_FULL, F = 2048, 256, 128
HEAD_LAG = 16
H = F
NCORES = 8
BS = B // NCORES  # 256 batch rows per core
WAVES = (("A", 0, 64), ("B", 64, 64), ("C", 128, 64), ("D", 192, 64))
W2 = 128
TC = 8            # timesteps per x-ingest chunk

# PSUM column layout (fp32 words per partition, 4096 total = 8 banks x 512)
PS_BUF = (0, 1024)     # two step buffers, 2 banks each (banks 0-3)
BLK = 256              # block width: [i|f|o|2g] each 256 cols (A:0-127 B:128+)
WLIN = {"A": 2048, "B": 2560, "C": 3072}  # banks 4/5/6, one per wave


def build(T=T_FULL, ablate=(), period_ns=0.0, waves=WAVES):
    """ablate: timing-only experiment knobs (results become wrong):
    'wlin' drop output-head matmuls; 'xin' drop x ingest (static xT);
    'dve' drop cell-math DVE ops; 'bias' drop bias matmuls; 'act' drop
    sigma ops; 'rec' drop recurrent matmuls.
    period_ns > 0 paces the scan at that many ns per step via virtual
    not-before times (tile_wait_until), phase-shifting the two waves."""
    ablate = set(ablate)
    assert T % 2 == 0
    nc = bacc.Bacc("TRN2", target_bir_lowering=False, debug=False,
                   num_devices=NCORES)

    xt_d = nc.dram_tensor("xt", [F, T, BS], BF16, kind="ExternalInput")
    whh_d = nc.dram_tensor("whh", [H, 4 * H], BF16, kind="ExternalInput")
    wih_d = nc.dram_tensor("wih", [F, 4 * H], BF16, kind="ExternalInput")
    b4_d = nc.dram_tensor("b4", [4, H], BF16, kind="ExternalInput")
    e4_d = nc.dram_tensor("e4", [4, 1024], BF16, kind="ExternalInput")
    wl_d = nc.dram_tensor("wl", [H, T], BF16, kind="ExternalInput")
    out_d = nc.dram_tensor("out", [BS], F32, kind="ExternalOutput")

    n_chunks = (T + TC - 1) // TC
    no_dve = "dve" in ablate
    no_act = "act" in ablate

    with tile.TileContext(nc) as tc:
        with (
            tc.tile_pool(name="const", bufs=1) as constp,
            tc.tile_pool(name="xT", bufs=3) as xtp,
            tc.tile_pool(name="sig", bufs=3) as sigp,
            tc.tile_pool(name="hh", bufs=20) as hhp,
            tc.tile_pool(name="cd", bufs=3) as cdp,
            tc.tile_pool(name="tmp", bufs=3) as tmpp,
            tc.tile_pool(name="psum", bufs=1, space=bass.MemorySpace.PSUM) as psp,
        ):
            # ---- constants ----
            whh = constp.tile([H, 4 * H], BF16)
            wih = constp.tile([F, 4 * H], BF16)
            b4 = constp.tile([4, H], BF16)
            e4 = constp.tile([4, 1024], BF16)
            wl = constp.tile([H, T], BF16)
            nc.sync.dma_start(whh[:], whh_d.ap())
            nc.sync.dma_start(wih[:], wih_d.ap())
            nc.sync.dma_start(b4[:], b4_d.ap())
            nc.sync.dma_start(e4[:], e4_d.ap())
            nc.sync.dma_start(wl[:], wl_d.ap())

            ps = psp.tile([128, 4096], F32)

            # ---- x ingest: host pre-transposed [F, T, BS] bf16, plain
            # chunked DMA straight into the matmul-ready layout ----
            xtap = xt_d.ap()
            xchunks = []
            if "xin" in ablate:
                x0 = constp.tile([F, TC, BS], BF16)
                nc.sync.dma_start(x0[:], xtap[:, 0:TC, :])
                xchunks = [x0] * n_chunks
            else:
                for ch in range(n_chunks):
                    t0 = ch * TC
                    tc_n = min(TC, T - t0)
                    xc = xtp.tile([F, TC, BS], BF16)
                    nc.sync.dma_start(xc[:, 0:tc_n, :], xtap[:, t0:t0 + tc_n, :])
                    xchunks.append(xc)

            def make_xt(t):
                return xchunks[t // TC][:, t % TC, :]

            # ---- state tiles ----
            # wlin accumulators: pack waves two-per-bank in banks 4-7. Each
            # bank is zeroed by one ones^T@zeros matmul (start=True) before
            # the scan; the RAW dep on those cols orders it before every
            # wlin accumulate (all start=False).
            wl_bank = {}
            used_banks = set()
            for i, (w, co, wn) in enumerate(waves):
                wl_bank[w] = 2048 + 256 * i
                used_banks.add(2048 + 512 * (i // 2))
            zrow = constp.tile([H, 512], BF16)
            onesc = constp.tile([H, 1], BF16)
            nc.vector.memset(zrow[:], 0.0)
            nc.vector.memset(onesc[:], 1.0)
            for bk in sorted(used_banks):
                nc.tensor.matmul(ps[0:1, bk:bk + 512], onesc[:], zrow[:],
                                 start=True, stop=False,
                                 skip_group_check=True)
            wv_w = {w: n for w, _, n in waves}
            cd_prev = {}
            for w, _, wn in waves:
                t0c = cdp.tile([H, wn], F32, tag=f"cd{w}")
                nc.vector.memset(t0c[:], 0.0)
                cd_prev[w] = t0c
            if no_dve:
                hh_s = {}
                for w, _, wn in waves:
                    hht = hhp.tile([H, wn], BF16, tag=f"hh{w}")
                    nc.vector.memset(hht[:], 0.01)
                    hh_s[w] = hht
            if no_act:
                s_s = {}
                for w, _, wn in waves:
                    sst = sigp.tile([128, 4, wn], BF16, tag=f"s{w}")
                    nc.vector.memset(sst[:], 0.5)
                    s_s[w] = sst
                scd_s = tmpp.tile([H, 256], BF16, tag="scds")
                nc.vector.memset(scd_s[:], 0.5)

            hh_prev = {w: None for w, _, _ in waves}
            hh_hist = {w: {} for w, _, _ in waves}

            def emit_head(t):
                """Output-head matmuls for step t, emitted LAG steps late so
                their hh dependency is long satisfied and the in-order PE
                queue never stalls on them."""
                if "wlin" in ablate or no_dve:
                    return
                for w, _, wn in waves:
                    hh = hh_hist[w].pop(t)
                    nc.tensor.matmul(
                        ps[0:1, wl_bank[w]:wl_bank[w] + wn],
                        wl[:, t:t + 1], hh[:],
                        start=False, stop=(t == T - 1),
                        skip_group_check=True)
            s_last = {w: None for w, _, _ in waves}
            wv_co = {w: c for w, c, _ in waves}

            cd_cur = {}

            def wave_p1(t, w):
                """Recurrent matmuls + the packed gates sigma."""
                base = PS_BUF[t % 2]
                co, wn = wv_co[w], wv_w[w]
                if hh_prev[w] is not None and "rec" not in ablate:
                    for c in range(4):
                        nc.tensor.matmul(
                            ps[:, base + c * BLK + co:base + c * BLK + co + wn],
                            whh[:, c * H:(c + 1) * H], hh_prev[w][:],
                            start=False, stop=False, skip_group_check=True)
                blocks = ps[:, base:base + 1024].rearrange(
                    "p (c n) -> p c n", c=4)
                if no_act:
                    s = s_s[w]
                else:
                    s = sigp.tile([128, 4, wn], BF16, tag=f"s{w}")
                    nc.scalar.activation(s[:], blocks[:, :, co:co + wn],
                                         AF.Sigmoid)
                s_last[w] = s

            def wave_p2(t, w):
                """Cell-state DVE chain: t2h, u, cd."""
                if no_dve:
                    return
                s = s_last[w]
                wn = wv_w[w]
                t2h = tmpp.tile([H, wn], BF16, tag=f"t2h{w}")
                nc.vector.scalar_tensor_tensor(
                    t2h[:], s[:, 3, :], -0.5, s[:, 0, :], OP.add, OP.mult)
                u = tmpp.tile([H, wn], F32, tag=f"u{w}")
                nc.vector.tensor_tensor(u[:], s[:, 1, :], cd_prev[w][:],
                                        OP.mult)
                cd = cdp.tile([H, wn], F32, tag=f"cd{w}")
                nc.vector.scalar_tensor_tensor(
                    cd[:], t2h[:], 4.0, u[:], OP.mult, OP.add)
                cd_prev[w] = cd
                cd_cur[w] = cd

            def wave_p3(t, w):
                """tanh(c) (via scaled-Tanh ACT, same table set as Sigmoid),
                h = tanh(c)*sgo as a plain 2x-mode tensor_tensor, and the
                output-head matmul."""
                if no_dve:
                    hh_prev[w] = hh_s[w]
                    return
                s = s_last[w]
                wn = wv_w[w]
                if no_act:
                    scd = scd_s[:, 0:wn]
                else:
                    scdt = tmpp.tile([H, wn], BF16, tag=f"scd{w}")
                    nc.scalar.activation(scdt[:], cd_cur[w][:], AF.Tanh,
                                         scale=0.5)
                    scd = scdt[:]
                hh = hhp.tile([H, wn], BF16, tag=f"hh{w}")
                nc.vector.tensor_tensor(hh[:], scd, s[:, 2, :], OP.mult)
                hh_prev[w] = hh
                hh_hist[w][t] = hh

            # ---- main scan: 3-phase wave rotation. Waves A/B/C run 1/3 step
            # apart so their ACT ops (sigma-gates, tanh-cd) round-robin on
            # the Activation engine without convoys. Steady-state slot t:
            #   xg(t) A.p1(t) C.p2(t-1) B.p3(t-1) B.p1(t) A.p2(t)
            #   C.p3(t-1) C.p1(t) B.p2(t) A.p3(t)
            first = "bias" in ablate

            def emit_xg(t):
                base = PS_BUF[t % 2]
                xt = make_xt(t)
                if "bias" not in ablate:
                    nc.tensor.matmul(
                        ps[:, base:base + 512], b4[:], e4[:, 0:512],
                        start=True, stop=False, skip_group_check=True)
                    nc.tensor.matmul(
                        ps[:, base + 512:base + 1024], b4[:],
                        e4[:, 512:1024],
                        start=True, stop=False, skip_group_check=True)
                for c in range(4):
                    nc.tensor.matmul(
                        ps[:, base + c * BLK:base + (c + 1) * BLK],
                        wih[:, c * H:(c + 1) * H], xt,
                        start=first, stop=False, skip_group_check=True)

            PH = {1: wave_p1, 2: wave_p2, 3: wave_p3}
            NW = len(waves)

            for slot in range(T + 1):
                if slot < T:
                    emit_xg(slot)
                ev = []
                for i, (w, _, _) in enumerate(waves):
                    for ph in (1, 2, 3):
                        # wave i runs phase ph of step t at virtual time
                        # t + i/NW + (ph-1)/3; emit the one landing in
                        # [slot, slot+1)
                        frac = i / NW + (ph - 1) / 3.0
                        t = slot - int(frac)
                        vt = frac - int(frac)
                        ev.append((vt, ph, t, w))
                ev.sort(key=lambda e: (e[0], -e[2]))
                for vt, ph, t, w in ev:
                    if 0 <= t < T:
                        PH[ph](t, w)
                if 0 <= slot - HEAD_LAG < T:
                    emit_head(slot - HEAD_LAG)
            for t in range(max(0, T + 1 - HEAD_LAG), T):
                emit_head(t)

            # output
            outsb = constp.tile([1, 2 * W2], F32)
            for w, co, wn in waves:
                nc.vector.tensor_copy(outsb[0:1, co:co + wn],
                                      ps[0:1, wl_bank[w]:wl_bank[w] + wn])
            nc.sync.dma_start(out_d.ap().rearrange("(a b) -> a b", a=1),
                              outsb[:])

    nc.compile()
    return nc


_CACHE = {}


def _get_nc(T=T_FULL):
    if T not in _CACHE:
        _CACHE[T] = build(T)
    return _CACHE[T]


def prep_weights(w_ih, w_hh, b_ih, b_hh, w_lin, T=T_FULL):
    """Host-side weight prep. Chunk order [i, f, o, g]; g-chunk pre-scaled x2
    (sigmoid(2g) trick). hh on device holds full h (tanh-based), so W_hh and
    w_lin are NOT pre-scaled."""
    perm = np.r_[0:H, H:2 * H, 3 * H:4 * H, 2 * H:3 * H]
    gs = np.ones((4 * H, 1), np.float32)
    gs[3 * H:] = 2.0
    bf = ml_dtypes.bfloat16
    whh = np.ascontiguousarray((w_hh[perm] * gs).T.astype(bf))
    wih = np.ascontiguousarray((w_ih[perm] * gs).T.astype(bf))
    b4 = ((b_ih + b_hh)[perm] * gs[:, 0]).reshape(4, H).astype(bf)
    e4 = np.zeros((4, 1024), bf)
    for c in range(4):
        e4[c, c * 256:(c + 1) * 256] = 1.0
    wl = np.ascontiguousarray(w_lin.reshape(T, H).T.astype(bf))
    return whh, wih, b4, e4, wl


def prep_x(x):
    """Shard + host-transpose x to [F, T, BS] bf16 per core (the layout the
    xg matmuls consume, so no on-chip transpose is needed)."""
    xb = x.astype(ml_dtypes.bfloat16)
    return [np.ascontiguousarray(xb[c * BS:(c + 1) * BS].transpose(2, 1, 0))
            for c in range(NCORES)]


def kernel(x, w_ih, w_hh, b_ih, b_hh, w_lin, b_lin):
    x = np.asarray(x, np.float32)
    T = x.shape[1]
    nc = _get_nc(T)
    whh, wih, b4, e4, wl = prep_weights(
        np.asarray(w_ih, np.float32), np.asarray(w_hh, np.float32),
        np.asarray(b_ih, np.float32), np.asarray(b_hh, np.float32),
        np.asarray(w_lin, np.float32), T)
    xts = prep_x(x)
    in_maps = []
    for c in range(NCORES):
        in_maps.append({
            "xt": xts[c],
            "whh": whh, "wih": wih, "b4": b4, "e4": e4, "wl": wl,
        })
    res = run_bass_kernel_spmd(nc, in_maps, core_ids=list(range(NCORES)))
    out = np.concatenate([r["out"] for r in res.results])
    return (out + np.float32(b_lin[0])).astype(np.float32)



# revision 6
# speedup vs baseline: 1.4449x; 1.1188x over previous
"""Trainium2 Bass kernel for nn_BiLSTM_2491081031886.

Single-layer unidirectional LSTM (B=2048, T=256, F=H=128) + Linear([T*H]->1).
Data-parallel over 8 NeuronCores: each core owns a 256-row batch shard and
runs the full sequential scan locally; weights are replicated.

Per-core dataflow (v7), all layouts [hidden, batch]:
  - x is pre-transposed and cast to bf16 on the host ([F, T, BS] per shard),
    so chunked plain DMA lands it directly in the matmul-ready layout.
  - Gate pre-activations accumulate in PSUM as 4 blocks [i|f|o|2g] x 256 cols
    per step, in two rotating 1024-col buffers (banks 0-3):
      bias (K=4 matmul vs a block-indicator, bf16)
      + W_ih^T.T @ xT_t (bf16, N=256)
      + W_hh^T.T @ h (bf16, N=64 per wave)
  - FOUR independent batch waves (64 cols each) run the recurrence with no
    cross-wave data dependency, interleaving on the engines to hide the
    per-step matmul->sigmoid->DVE dependency-ring latency. Per wave/step:
      sg  = Sigmoid(blocks)              # one packed ACT op, bf16 out
      t2h = (sg2g - 0.5) * sgi           # DVE stt, bf16
      u   = sgf * cd_prev                # DVE tt (cd fp32 SBUF)
      cd  = 4*t2h + u                    # DVE stt, fp32
      th  = Tanh(0.5 * cd)               # ACT (same table set as Sigmoid)
      h   = th * sgo                     # DVE tt, bf16 (2x mode)
    with tanh(g) via sigmoid (g-chunk pre-scaled x2, doubled cell state
    cd = 2c) and tanh(c) via the free-scale Tanh op, so h is the full
    hidden state (no weight rescale).
  - All waves write h into ONE shared per-step tile (ring of 20), read back
    per-wave as slices by the recurrent matmuls.
  - Output head: ONE [1,256] PE matmul per step (lhsT = w_lin column),
    accumulated in PSUM bank 6, emitted HEAD_LAG=16 steps LATE so its h
    dependency is long satisfied -- the in-order PE queue never stalls on
    it (head-of-line blocking was worth ~20% on HW). The accumulator bank
    is zeroed once by a ones^T@zeros start=True matmul whose RAW dep
    orders it before all start=False accumulates; +b_lin on host.
"""

import numpy as np
import ml_dtypes

import concourse.bacc as bacc
import concourse.bass as bass
import concourse.mybir as mybir
from concourse import tile
from concourse.bass_utils import run_bass_kernel_spmd

F32 = mybir.dt.float32
BF16 = mybir.dt.bfloat16
AF = mybir.ActivationFunctionType
OP = mybir.AluOpType

B, T_FULL, F = 2048, 256, 128
HEAD_LAG = 16
HEAD_PS = 3072  # bank 6: single [1, 256] output-head accumulator
H = F
NCORES = 8
BS = B // NCORES  # 256 batch rows per core
WAVES = (("A", 0, 64), ("B", 64, 64), ("C", 128, 64), ("D", 192, 64))
W2 = 128
TC = 8            # timesteps per x-ingest chunk

# PSUM column layout (fp32 words per partition, 4096 total = 8 banks x 512)
PS_BUF = (0, 1024)     # two step buffers, 2 banks each (banks 0-3)
BLK = 256              # block width: [i|f|o|2g] each 256 cols (A:0-127 B:128+)
WLIN = {"A": 2048, "B": 2560, "C": 3072}  # banks 4/5/6, one per wave


def build(T=T_FULL, ablate=(), period_ns=0.0, waves=WAVES):
    """ablate: timing-only experiment knobs (results become wrong):
    'wlin' drop output-head matmuls; 'xin' drop x ingest (static xT);
    'dve' drop cell-math DVE ops; 'bias' drop bias matmuls; 'act' drop
    sigma ops; 'rec' drop recurrent matmuls.
    period_ns > 0 paces the scan at that many ns per step via virtual
    not-before times (tile_wait_until), phase-shifting the two waves."""
    ablate = set(ablate)
    assert T % 2 == 0
    nc = bacc.Bacc("TRN2", target_bir_lowering=False, debug=False,
                   num_devices=NCORES)

    xt_d = nc.dram_tensor("xt", [F, T, BS], BF16, kind="ExternalInput")
    whh_d = nc.dram_tensor("whh", [H, 4 * H], BF16, kind="ExternalInput")
    wih_d = nc.dram_tensor("wih", [F, 4 * H], BF16, kind="ExternalInput")
    b4_d = nc.dram_tensor("b4", [4, H], BF16, kind="ExternalInput")
    e4_d = nc.dram_tensor("e4", [4, 1024], BF16, kind="ExternalInput")
    wl_d = nc.dram_tensor("wl", [H, T], BF16, kind="ExternalInput")
    out_d = nc.dram_tensor("out", [BS], F32, kind="ExternalOutput")

    n_chunks = (T + TC - 1) // TC
    no_dve = "dve" in ablate
    no_act = "act" in ablate

    with tile.TileContext(nc) as tc:
        with (
            tc.tile_pool(name="const", bufs=1) as constp,
            tc.tile_pool(name="xT", bufs=3) as xtp,
            tc.tile_pool(name="sig", bufs=3) as sigp,
            tc.tile_pool(name="hh", bufs=20) as hhp,
            tc.tile_pool(name="cd", bufs=3) as cdp,
            tc.tile_pool(name="tmp", bufs=3) as tmpp,
            tc.tile_pool(name="psum", bufs=1, space=bass.MemorySpace.PSUM) as psp,
        ):
            # ---- constants ----
            whh = constp.tile([H, 4 * H], BF16)
            wih = constp.tile([F, 4 * H], BF16)
            b4 = constp.tile([4, H], BF16)
            e4 = constp.tile([4, 1024], BF16)
            wl = constp.tile([H, T], BF16)
            nc.sync.dma_start(whh[:], whh_d.ap())
            nc.sync.dma_start(wih[:], wih_d.ap())
            nc.sync.dma_start(b4[:], b4_d.ap())
            nc.sync.dma_start(e4[:], e4_d.ap())
            nc.sync.dma_start(wl[:], wl_d.ap())

            ps = psp.tile([128, 4096], F32)

            # ---- x ingest: host pre-transposed [F, T, BS] bf16, plain
            # chunked DMA straight into the matmul-ready layout ----
            xtap = xt_d.ap()
            xchunks = []
            if "xin" in ablate:
                x0 = constp.tile([F, TC, BS], BF16)
                nc.sync.dma_start(x0[:], xtap[:, 0:TC, :])
                xchunks = [x0] * n_chunks
            else:
                for ch in range(n_chunks):
                    t0 = ch * TC
                    tc_n = min(TC, T - t0)
                    xc = xtp.tile([F, TC, BS], BF16)
                    nc.sync.dma_start(xc[:, 0:tc_n, :], xtap[:, t0:t0 + tc_n, :])
                    xchunks.append(xc)

            def make_xt(t):
                return xchunks[t // TC][:, t % TC, :]

            # ---- state tiles ----
            # single output-head accumulator [1, 256] in bank 6, zeroed
            # by one ones^T@zeros matmul (start=True) before the scan; the
            # RAW dep on those cols orders it before every head accumulate
            # (all start=False).
            zrow = constp.tile([H, 512], BF16)
            onesc = constp.tile([H, 1], BF16)
            nc.vector.memset(zrow[:], 0.0)
            nc.vector.memset(onesc[:], 1.0)
            nc.tensor.matmul(ps[0:1, HEAD_PS:HEAD_PS + 512], onesc[:],
                             zrow[:], start=True, stop=False,
                             skip_group_check=True)
            wv_w = {w: n for w, _, n in waves}
            cd_prev = {}
            for w, _, wn in waves:
                t0c = cdp.tile([H, wn], F32, tag=f"cd{w}")
                nc.vector.memset(t0c[:], 0.0)
                cd_prev[w] = t0c
            if no_dve:
                hh_s = {}
                for w, _, wn in waves:
                    hht = hhp.tile([H, wn], BF16, tag=f"hh{w}")
                    nc.vector.memset(hht[:], 0.01)
                    hh_s[w] = hht
            if no_act:
                s_s = {}
                for w, _, wn in waves:
                    sst = sigp.tile([128, 4, wn], BF16, tag=f"s{w}")
                    nc.vector.memset(sst[:], 0.5)
                    s_s[w] = sst
                scd_s = tmpp.tile([H, 256], BF16, tag="scds")
                nc.vector.memset(scd_s[:], 0.5)

            hh_prev = {w: None for w, _, _ in waves}
            hh_step = {}

            def emit_head(t):
                """One output-head matmul for step t over ALL waves' hh,
                emitted LAG steps late so its hh dependencies are long
                satisfied and the in-order PE queue never stalls on it."""
                if "wlin" in ablate or no_dve:
                    return
                hht = hh_step.pop(t)
                nc.tensor.matmul(
                    ps[0:1, HEAD_PS:HEAD_PS + 2 * W2],
                    wl[:, t:t + 1], hht[:],
                    start=False, stop=(t == T - 1),
                    skip_group_check=True)
            s_last = {w: None for w, _, _ in waves}
            wv_co = {w: c for w, c, _ in waves}

            cd_cur = {}

            def wave_p1(t, w):
                """Recurrent matmuls + the packed gates sigma."""
                base = PS_BUF[t % 2]
                co, wn = wv_co[w], wv_w[w]
                if hh_prev[w] is not None and "rec" not in ablate:
                    for c in range(4):
                        nc.tensor.matmul(
                            ps[:, base + c * BLK + co:base + c * BLK + co + wn],
                            whh[:, c * H:(c + 1) * H], hh_prev[w][:],
                            start=False, stop=False, skip_group_check=True)
                blocks = ps[:, base:base + 1024].rearrange(
                    "p (c n) -> p c n", c=4)
                if no_act:
                    s = s_s[w]
                else:
                    s = sigp.tile([128, 4, wn], BF16, tag=f"s{w}")
                    nc.scalar.activation(s[:], blocks[:, :, co:co + wn],
                                         AF.Sigmoid)
                s_last[w] = s

            def wave_p2(t, w):
                """Cell-state DVE chain: t2h, u, cd."""
                if no_dve:
                    return
                s = s_last[w]
                wn = wv_w[w]
                t2h = tmpp.tile([H, wn], BF16, tag=f"t2h{w}")
                nc.vector.scalar_tensor_tensor(
                    t2h[:], s[:, 3, :], -0.5, s[:, 0, :], OP.add, OP.mult)
                u = tmpp.tile([H, wn], F32, tag=f"u{w}")
                nc.vector.tensor_tensor(u[:], s[:, 1, :], cd_prev[w][:],
                                        OP.mult)
                cd = cdp.tile([H, wn], F32, tag=f"cd{w}")
                nc.vector.scalar_tensor_tensor(
                    cd[:], t2h[:], 4.0, u[:], OP.mult, OP.add)
                cd_prev[w] = cd
                cd_cur[w] = cd

            def wave_p3(t, w):
                """tanh(c) (via scaled-Tanh ACT, same table set as Sigmoid),
                h = tanh(c)*sgo as a plain 2x-mode tensor_tensor, and the
                output-head matmul."""
                if no_dve:
                    hh_prev[w] = hh_s[w]
                    return
                s = s_last[w]
                wn = wv_w[w]
                if no_act:
                    scd = scd_s[:, 0:wn]
                else:
                    scdt = tmpp.tile([H, wn], BF16, tag=f"scd{w}")
                    nc.scalar.activation(scdt[:], cd_cur[w][:], AF.Tanh,
                                         scale=0.5)
                    scd = scdt[:]
                co = wv_co[w]
                if t not in hh_step:
                    hht = hhp.tile([H, 2 * W2], BF16, tag="hh")
                    hh_step[t] = hht
                hh = hh_step[t][:, co:co + wn]
                nc.vector.tensor_tensor(hh, scd, s[:, 2, :], OP.mult)
                hh_prev[w] = hh

            # ---- main scan: 3-phase wave rotation. Waves A/B/C run 1/3 step
            # apart so their ACT ops (sigma-gates, tanh-cd) round-robin on
            # the Activation engine without convoys. Steady-state slot t:
            #   xg(t) A.p1(t) C.p2(t-1) B.p3(t-1) B.p1(t) A.p2(t)
            #   C.p3(t-1) C.p1(t) B.p2(t) A.p3(t)
            first = "bias" in ablate

            def emit_xg(t):
                base = PS_BUF[t % 2]
                xt = make_xt(t)
                if "bias" not in ablate:
                    nc.tensor.matmul(
                        ps[:, base:base + 512], b4[:], e4[:, 0:512],
                        start=True, stop=False, skip_group_check=True)
                    nc.tensor.matmul(
                        ps[:, base + 512:base + 1024], b4[:],
                        e4[:, 512:1024],
                        start=True, stop=False, skip_group_check=True)
                for c in range(4):
                    nc.tensor.matmul(
                        ps[:, base + c * BLK:base + (c + 1) * BLK],
                        wih[:, c * H:(c + 1) * H], xt,
                        start=first, stop=False, skip_group_check=True)

            PH = {1: wave_p1, 2: wave_p2, 3: wave_p3}
            NW = len(waves)

            for slot in range(T + 1):
                if slot < T:
                    emit_xg(slot)
                ev = []
                for i, (w, _, _) in enumerate(waves):
                    for ph in (1, 2, 3):
                        # wave i runs phase ph of step t at virtual time
                        # t + i/NW + (ph-1)/3; emit the one landing in
                        # [slot, slot+1)
                        frac = i / NW + (ph - 1) / 3.0
                        t = slot - int(frac)
                        vt = frac - int(frac)
                        ev.append((vt, ph, t, w))
                ev.sort(key=lambda e: (e[0], -e[2]))
                for vt, ph, t, w in ev:
                    if 0 <= t < T:
                        PH[ph](t, w)
                if 0 <= slot - HEAD_LAG < T:
                    emit_head(slot - HEAD_LAG)
            for t in range(max(0, T + 1 - HEAD_LAG), T):
                emit_head(t)

            # output
            outsb = constp.tile([1, 2 * W2], F32)
            nc.vector.tensor_copy(outsb[0:1, 0:2 * W2],
                                  ps[0:1, HEAD_PS:HEAD_PS + 2 * W2])
            nc.sync.dma_start(out_d.ap().rearrange("(a b) -> a b", a=1),
                              outsb[:])

    nc.compile()
    return nc


_CACHE = {}


def _get_nc(T=T_FULL):
    if T not in _CACHE:
        _CACHE[T] = build(T)
    return _CACHE[T]


def prep_weights(w_ih, w_hh, b_ih, b_hh, w_lin, T=T_FULL):
    """Host-side weight prep. Chunk order [i, f, o, g]; g-chunk pre-scaled x2
    (sigmoid(2g) trick). hh on device holds full h (tanh-based), so W_hh and
    w_lin are NOT pre-scaled."""
    perm = np.r_[0:H, H:2 * H, 3 * H:4 * H, 2 * H:3 * H]
    gs = np.ones((4 * H, 1), np.float32)
    gs[3 * H:] = 2.0
    bf = ml_dtypes.bfloat16
    whh = np.ascontiguousarray((w_hh[perm] * gs).T.astype(bf))
    wih = np.ascontiguousarray((w_ih[perm] * gs).T.astype(bf))
    b4 = ((b_ih + b_hh)[perm] * gs[:, 0]).reshape(4, H).astype(bf)
    e4 = np.zeros((4, 1024), bf)
    for c in range(4):
        e4[c, c * 256:(c + 1) * 256] = 1.0
    wl = np.ascontiguousarray(w_lin.reshape(T, H).T.astype(bf))
    return whh, wih, b4, e4, wl


def prep_x(x):
    """Shard + host-transpose x to [F, T, BS] bf16 per core (the layout the
    xg matmuls consume, so no on-chip transpose is needed)."""
    xb = x.astype(ml_dtypes.bfloat16)
    return [np.ascontiguousarray(xb[c * BS:(c + 1) * BS].transpose(2, 1, 0))
            for c in range(NCORES)]


def kernel(x, w_ih, w_hh, b_ih, b_hh, w_lin, b_lin):
    x = np.asarray(x, np.float32)
    T = x.shape[1]
    nc = _get_nc(T)
    whh, wih, b4, e4, wl = prep_weights(
        np.asarray(w_ih, np.float32), np.asarray(w_hh, np.float32),
        np.asarray(b_ih, np.float32), np.asarray(b_hh, np.float32),
        np.asarray(w_lin, np.float32), T)
    xts = prep_x(x)
    in_maps = []
    for c in range(NCORES):
        in_maps.append({
            "xt": xts[c],
            "whh": whh, "wih": wih, "b4": b4, "e4": e4, "wl": wl,
        })
    res = run_bass_kernel_spmd(nc, in_maps, core_ids=list(range(NCORES)))
    out = np.concatenate([r["out"] for r in res.results])
    return (out + np.float32(b_lin[0])).astype(np.float32)

